# revision 1
# baseline (speedup 1.0000x reference)
"""Bass kernel for the Balloon-Windkessel BOLD layer (trn2, 8 cores).

Layout: time-on-partitions (T=128 rows per block), regions on free dim.
Per core: R regions (250), NB blocks of T steps (time padded to NB*T).

Per block (old-aligned XO,FO,VO; outputs new-aligned):
  XO/FO : bf16-split-3 matmuls over Z + f32 1-row carry matmuls
  XN/FN : elementwise from XO,FO,Z
  v     : strict-cumsum base + ramp guess + M delta-Picard iterations
          (iteration deltas in bf16, accumulated in one PSUM group)
  q     : exact product-form solve (poly ln(alpha), exp cumsums, split-2)
  bold  : elementwise; 1/v, 1/f via DVE reciprocal
"""
import numpy as np
import ml_dtypes

import concourse.bacc as bacc
import concourse.mybir as mybir
from concourse.tile import TileContext

F32 = mybir.dt.float32
BF16 = mybir.dt.bfloat16
NPBF = ml_dtypes.bfloat16
AF = mybir.ActivationFunctionType
OP = mybir.AluOpType

KAPPA, GAMMAB, TAO, ALPHA, RO = 0.65, 0.41, 0.98, 0.32, 0.34
V_0, K_1, K_2, K_3 = 0.02, 2.38, 2.0, 0.48


def host_consts(T, dt):
    """All block constants, computed in float64, split for the PE."""
    a = 1.0 - dt * KAPPA
    b = dt * GAMMAB
    c = dt / TAO
    M2 = np.array([[a, -b], [dt, 1.0]])
    A = np.zeros((T + 1, 2, 2))
    A[0] = np.eye(2)
    for p in range(1, T + 1):
        A[p] = M2 @ A[p - 1]
    LXZ = np.zeros((T, T))
    LFZ = np.zeros((T, T))
    for p in range(T):
        for j in range(p):
            LXZ[j, p] = dt * A[p - 1 - j, 0, 0]
            LFZ[j, p] = dt * A[p - 1 - j, 1, 0]
    cum11 = np.cumsum(A[:T, 0, 0])
    cum21 = np.cumsum(A[:T, 1, 0])
    cxo0 = np.zeros(T)
    cfo0 = np.zeros(T)
    cxo0[1:] = b * cum11[:T - 1]
    cfo0[1:] = b * cum21[:T - 1]

    def bs(x):
        hi = x.astype(NPBF)
        lo = (x - hi.astype(np.float64)).astype(NPBF)
        return hi, lo

    LXZh, LXZl = bs(LXZ)
    LFZh, LFZl = bs(LFZ)
    arows = np.zeros((7, T), np.float32)
    arows[0] = A[:T, 0, 0]
    arows[1] = A[:T, 0, 1]
    arows[2] = A[:T, 1, 0]
    arows[3] = A[:T, 1, 1]
    arows[4] = -np.arange(T) / c          # ramp (times gc row, scaled by c later)
    arows[5] = 1.0 / c                    # vc broadcast (scaled by c later)
    arows[6] = 1.0                        # qc broadcast
    cols = np.zeros((T, 3), np.float32)
    cols[:, 0] = a * cxo0 + b
    cols[:, 1] = dt * cxo0
    cols[:, 2] = cfo0
    return dict(
        LXZh=LXZh, LXZl=LXZl, LFZh=LFZh, LFZl=LFZl,
        SS01=np.triu(np.ones((T, T)), 1).astype(NPBF),
        SI01=np.triu(np.ones((T, T)), 0).astype(NPBF),
        NSI01=(-np.triu(np.ones((T, T)), 0)).astype(NPBF),
        arows=arows, cols=cols,
    )


def make_inputs(z_shard, init_shard, T, NB, dt):
    """Build the per-core in_map. z_shard [NS, R] f32, init_shard [R, 4]."""
    NS, R = z_shard.shape
    c = dt / TAO
    ia = 1.0 / ALPHA
    z = np.zeros((NB * T, R), np.float32)
    z[:NS] = z_shard
    C = host_consts(T, dt)
    init4 = np.ascontiguousarray(init_shard.T.astype(np.float32))  # [4, R]
    gc0 = (c * np.exp(ia * np.log(init4[2].astype(np.float64)))
           ).astype(np.float32)[None, :]
    return {
        "z": z, "init4": init4, "gc0": gc0,
        "lxzh": C["LXZh"], "lxzl": C["LXZl"],
        "lfzh": C["LFZh"], "lfzl": C["LFZl"],
        "ss01": C["SS01"], "si01": C["SI01"], "nsi01": C["NSI01"],
        "arows": C["arows"], "cols": C["cols"],
    }


def build_kernel(R, T, NB, M, dt):
    c = dt / TAO
    ia = 1.0 / ALPHA
    lnc = float(np.log(c))
    lncRO = float(np.log(c / RO))
    lnrho = float(np.log(1.0 - RO))
    a = 1.0 - dt * KAPPA
    b = dt * GAMMAB
    k1, k2, k3 = V_0 * K_1, V_0 * K_2, V_0 * K_3
    CB = V_0 * (K_1 + K_2 + K_3)

    nc = bacc.Bacc("TRN2", target_bir_lowering=False, debug=False)
    z_d = nc.dram_tensor("z", [NB * T, R], F32, kind="ExternalInput")
    init_d = nc.dram_tensor("init4", [4, R], F32, kind="ExternalInput")
    gc0_d = nc.dram_tensor("gc0", [1, R], F32, kind="ExternalInput")
    lxzh_d = nc.dram_tensor("lxzh", [T, T], BF16, kind="ExternalInput")
    lxzl_d = nc.dram_tensor("lxzl", [T, T], BF16, kind="ExternalInput")
    lfzh_d = nc.dram_tensor("lfzh", [T, T], BF16, kind="ExternalInput")
    lfzl_d = nc.dram_tensor("lfzl", [T, T], BF16, kind="ExternalInput")
    ss_d = nc.dram_tensor("ss01", [T, T], BF16, kind="ExternalInput")
    si_d = nc.dram_tensor("si01", [T, T], BF16, kind="ExternalInput")
    nsi_d = nc.dram_tensor("nsi01", [T, T], BF16, kind="ExternalInput")
    ar_d = nc.dram_tensor("arows", [7, T], F32, kind="ExternalInput")
    cols_d = nc.dram_tensor("cols", [T, 3], F32, kind="ExternalInput")
    hist_d = nc.dram_tensor("hist", [NB * T, R * 5], F32, kind="ExternalOutput")

    with TileContext(nc) as tc:
        with (
            tc.tile_pool(name="cst", bufs=1) as cst,
            tc.tile_pool(name="zp", bufs=3) as zp,
            tc.tile_pool(name="wk", bufs=2) as wk,
            tc.tile_pool(name="gh", bufs=3) as ghp,
            tc.tile_pool(name="op", bufs=3) as op,
            tc.tile_pool(name="ps", bufs=1, space="PSUM") as ps,
        ):
            LXZh = cst.tile([T, T], BF16)
            LXZl = cst.tile([T, T], BF16)
            LFZh = cst.tile([T, T], BF16)
            LFZl = cst.tile([T, T], BF16)
            SS = cst.tile([T, T], BF16)
            SI = cst.tile([T, T], BF16)
            NSI = cst.tile([T, T], BF16)
            AR = cst.tile([7, T], F32)
            COLS = cst.tile([T, 3], F32)
            INIT = cst.tile([4, R], F32)
            GC0 = cst.tile([1, R], F32)
            for t_, d_ in ((LXZh, lxzh_d), (LXZl, lxzl_d), (LFZh, lfzh_d),
                           (LFZl, lfzl_d), (SS, ss_d), (SI, si_d),
                           (NSI, nsi_d), (AR, ar_d), (COLS, cols_d),
                           (INIT, init_d), (GC0, gc0_d)):
                nc.sync.dma_start(t_[:, :], d_[:, :])

            xc = INIT[0:1, :]
            fc = INIT[1:2, :]
            vc = INIT[2:3, :]
            qc = INIT[3:4, :]
            gc = GC0[0:1, :]

            for k in range(NB):
                Z = zp.tile([T, R], F32, tag="z")
                nc.sync.dma_start(Z[:, :], z_d[k * T:(k + 1) * T, :])
                Zh = zp.tile([T, R], BF16, tag="zh")
                nc.gpsimd.tensor_copy(Zh[:, :], Z[:, :])
                Zl = zp.tile([T, R], BF16, tag="zl")
                nc.vector.scalar_tensor_tensor(
                    Zl[:, :], Zh[:, :], -1.0, Z[:, :], OP.mult, OP.add)

                PX = ps.tile([T, R], F32, tag="px")
                nc.tensor.matmul(PX[:, :], LXZh[:, :], Zh[:, :], start=True, stop=False)
                nc.tensor.matmul(PX[:, :], LXZh[:, :], Zl[:, :], start=False, stop=False)
                nc.tensor.matmul(PX[:, :], LXZl[:, :], Zh[:, :], start=False, stop=False)
                nc.tensor.matmul(PX[:, :], AR[0:1, :], xc, start=False, stop=False)
                nc.tensor.matmul(PX[:, :], AR[1:2, :], fc, start=False, stop=True)
                PF = ps.tile([T, R], F32, tag="pf")
                nc.tensor.matmul(PF[:, :], LFZh[:, :], Zh[:, :], start=True, stop=False)
                nc.tensor.matmul(PF[:, :], LFZh[:, :], Zl[:, :], start=False, stop=False)
                nc.tensor.matmul(PF[:, :], LFZl[:, :], Zh[:, :], start=False, stop=False)
                nc.tensor.matmul(PF[:, :], AR[2:3, :], xc, start=False, stop=False)
                nc.tensor.matmul(PF[:, :], AR[3:4, :], fc, start=False, stop=True)

                FOs = wk.tile([T, R], F32, tag="fos")
                nc.scalar.activation(FOs[:, :], PF[:, :], AF.Identity,
                                     bias=COLS[:, 2:3], scale=1.0)

                OUT = op.tile([T, R * 5], F32, tag="out")
                O3 = OUT[:, :].rearrange("p (r c) -> p r c", c=5)
                t0 = wk.tile([T, R], F32, tag="t0")
                nc.scalar.activation(t0[:, :], Z[:, :], AF.Identity,
                                     bias=COLS[:, 0:1], scale=dt)
                t1 = wk.tile([T, R], F32, tag="t1")
                nc.vector.scalar_tensor_tensor(
                    t1[:, :], FOs[:, :], -b, t0[:, :], OP.mult, OP.add)
                nc.vector.scalar_tensor_tensor(
                    O3[:, :, 0], PX[:, :], a, t1[:, :], OP.mult, OP.add)
                t3 = wk.tile([T, R], F32, tag="t3")
                nc.scalar.activation(t3[:, :], PX[:, :], AF.Identity,
                                     bias=COLS[:, 1:2], scale=dt)
                nc.gpsimd.tensor_tensor(O3[:, :, 1], t3[:, :], FOs[:, :], OP.add)

                FOh = zp.tile([T, R], BF16, tag="foh")
                nc.gpsimd.tensor_copy(FOh[:, :], FOs[:, :])
                FOl = zp.tile([T, R], BF16, tag="fol")
                nc.vector.scalar_tensor_tensor(
                    FOl[:, :], FOh[:, :], -1.0, FOs[:, :], OP.mult, OP.add)
                PB = ps.tile([T, R], F32, tag="pb")
                nc.tensor.matmul(PB[:, :], SS[:, :], FOh[:, :], start=True, stop=False)
                nc.tensor.matmul(PB[:, :], SS[:, :], FOl[:, :], start=False, stop=False)
                nc.tensor.matmul(PB[:, :], AR[5:6, :], vc, start=False, stop=False)
                base_sb = wk.tile([T, R], F32, tag="base")
                nc.scalar.mul(base_sb[:, :], PB[:, :], c)
                nc.tensor.matmul(PB[:, :], AR[4:5, :], gc, start=False, stop=True)

                lnV = wk.tile([T, R], F32, tag="lnv")
                nc.scalar.activation(lnV[:, :], PB[:, :], AF.Log, scale=c)
                GHprev = None
                PG = None
                VO = None
                for m in range(M):
                    GH = ghp.tile([T, R], F32, tag="gh")
                    nc.scalar.activation(GH[:, :], lnV[:, :], AF.Exp,
                                         bias=lnc, scale=ia)
                    if m == 0:
                        GHh = zp.tile([T, R], BF16, tag="ghh")
                        nc.gpsimd.tensor_copy(GHh[:, :], GH[:, :])
                        GHl = zp.tile([T, R], BF16, tag="ghl")
                        nc.vector.scalar_tensor_tensor(
                            GHl[:, :], GHh[:, :], -1.0, GH[:, :], OP.mult, OP.add)
                        PG = ps.tile([T, R], F32, tag="pg")
                        nc.tensor.matmul(PG[:, :], SS[:, :], GHh[:, :],
                                         start=True, stop=False)
                        nc.tensor.matmul(PG[:, :], SS[:, :], GHl[:, :],
                                         start=False, stop=(M == 1))
                    else:
                        D = zp.tile([T, R], BF16, tag="d")
                        nc.vector.scalar_tensor_tensor(
                            D[:, :], GHprev[:, :], -1.0, GH[:, :], OP.mult, OP.add)
                        nc.tensor.matmul(PG[:, :], SS[:, :], D[:, :],
                                         start=False, stop=(m == M - 1))
                    GHprev = GH
                    VO = wk.tile([T, R], F32, tag="vo")
                    nc.vector.scalar_tensor_tensor(
                        VO[:, :], PG[:, :], -1.0, base_sb[:, :], OP.mult, OP.add)
                    if m < M - 1:
                        lnV = wk.tile([T, R], F32, tag="lnv")
                        nc.scalar.activation(lnV[:, :], VO[:, :], AF.Log)

                tv = wk.tile([T, R], F32, tag="tv")
                nc.vector.scalar_tensor_tensor(
                    tv[:, :], GHprev[:, :], -1.0, VO[:, :], OP.mult, OP.add)
                nc.vector.scalar_tensor_tensor(
                    O3[:, :, 2], FOs[:, :], c, tv[:, :], OP.mult, OP.add)

                Y = wk.tile([T, R], F32, tag="y")
                nc.scalar.activation(Y[:, :], lnV[:, :], AF.Exp,
                                     bias=lnc, scale=ia - 1.0)
                s1 = wk.tile([T, R], F32, tag="s1")
                nc.vector.tensor_scalar(s1[:, :], Y[:, :], 1.0 / 3.0, 0.5,
                                        OP.mult, OP.add)
                s2 = wk.tile([T, R], F32, tag="s2")
                nc.gpsimd.tensor_tensor(s2[:, :], Y[:, :], s1[:, :], OP.mult)
                s3 = wk.tile([T, R], F32, tag="s3")
                nc.vector.scalar_tensor_tensor(
                    s3[:, :], s2[:, :], 1.0, Y[:, :], OP.add, OP.mult)
                s3h = zp.tile([T, R], BF16, tag="s3h")
                nc.gpsimd.tensor_copy(s3h[:, :], s3[:, :])
                s3l = zp.tile([T, R], BF16, tag="s3l")
                nc.vector.scalar_tensor_tensor(
                    s3l[:, :], s3h[:, :], -1.0, s3[:, :], OP.mult, OP.add)
                PS = ps.tile([T, R], F32, tag="psq")
                nc.tensor.matmul(PS[:, :], NSI[:, :], s3h[:, :], start=True, stop=False)
                nc.tensor.matmul(PS[:, :], NSI[:, :], s3l[:, :], start=False, stop=True)
                P_sb = wk.tile([T, R], F32, tag="p")
                nc.scalar.activation(P_sb[:, :], PS[:, :], AF.Exp)
                E_sb = wk.tile([T, R], F32, tag="e")
                nc.scalar.activation(E_sb[:, :], PS[:, :], AF.Exp,
                                     bias=lncRO, scale=-1.0)
                RF = wk.tile([T, R], F32, tag="rfq")
                nc.vector.reciprocal(RF[:, :], FOs[:, :])
                PW = wk.tile([T, R], F32, tag="pw")
                nc.scalar.activation(PW[:, :], RF[:, :], AF.Exp, scale=lnrho)
                tq = wk.tile([T, R], F32, tag="tq")
                nc.gpsimd.tensor_tensor(tq[:, :], FOs[:, :], PW[:, :], OP.mult)
                Rm = wk.tile([T, R], F32, tag="rmq")
                nc.vector.scalar_tensor_tensor(
                    Rm[:, :], tq[:, :], -1.0, FOs[:, :], OP.mult, OP.add)
                RMf = wk.tile([T, R], F32, tag="rmf")
                nc.gpsimd.tensor_tensor(RMf[:, :], Rm[:, :], E_sb[:, :], OP.mult)
                RMh = zp.tile([T, R], BF16, tag="rmh")
                nc.gpsimd.tensor_copy(RMh[:, :], RMf[:, :])
                RMl = zp.tile([T, R], BF16, tag="rml")
                nc.vector.scalar_tensor_tensor(
                    RMl[:, :], RMh[:, :], -1.0, RMf[:, :], OP.mult, OP.add)
                PC = ps.tile([T, R], F32, tag="pc")
                nc.tensor.matmul(PC[:, :], SI[:, :], RMh[:, :], start=True, stop=False)
                nc.tensor.matmul(PC[:, :], SI[:, :], RMl[:, :], start=False, stop=False)
                nc.tensor.matmul(PC[:, :], AR[6:7, :], qc, start=False, stop=True)
                nc.vector.tensor_tensor(O3[:, :, 3], P_sb[:, :], PC[:, :], OP.mult)

                RV = wk.tile([T, R], F32, tag="rvb")
                nc.vector.reciprocal(RV[:, :], O3[:, :, 2])
                t2 = wk.tile([T, R], F32, tag="t2")
                nc.vector.scalar_tensor_tensor(
                    t2[:, :], RV[:, :], k2, O3[:, :, 3], OP.mult, OP.mult)
                w1 = wk.tile([T, R], F32, tag="w1")
                nc.vector.scalar_tensor_tensor(
                    w1[:, :], O3[:, :, 3], k1, t2[:, :], OP.mult, OP.add)
                w2 = wk.tile([T, R], F32, tag="w2")
                nc.vector.scalar_tensor_tensor(
                    w2[:, :], O3[:, :, 2], k3, w1[:, :], OP.mult, OP.add)
                nc.scalar.activation(O3[:, :, 4], w2[:, :], AF.Identity,
                                     bias=CB, scale=-1.0)

                nc.sync.dma_start(hist_d[k * T:(k + 1) * T, :], OUT[:, :])

                xc = O3[T - 1:T, :, 0]
                fc = O3[T - 1:T, :, 1]
                vc = O3[T - 1:T, :, 2]
                qc = O3[T - 1:T, :, 3]
                gc = GHprev[T - 1:T, :]
    nc.compile()
    return nc


_BUILD_CACHE = {}


def _get_kernel(R, T, NB, M, dt):
    key = (R, T, NB, M, dt)
    if key not in _BUILD_CACHE:
        _BUILD_CACHE[key] = build_kernel(R, T, NB, M, dt)
    return _BUILD_CACHE[key]


def kernel(init_state, node_history, step_size, sim_len, _trace=False):
    """Full-input entry point: shards regions over 8 neuron cores."""
    from concourse.bass_utils import run_bass_kernel_spmd

    init_state = np.asarray(init_state, dtype=np.float32)
    node_history = np.asarray(node_history, dtype=np.float32)
    step_size = int(step_size)
    sim_len = int(sim_len)
    NS = sim_len // step_size
    assert node_history.shape[0] == NS
    NR = node_history.shape[1]
    NCORES = 8
    assert NR % NCORES == 0
    R = NR // NCORES
    T = 128
    NB = (NS + T - 1) // T
    M = 4
    dt = step_size / 1000.0

    nc = _get_kernel(R, T, NB, M, dt)
    in_maps = []
    for cix in range(NCORES):
        r0 = cix * R
        z_shard = np.ascontiguousarray(node_history[:, r0:r0 + R])
        init_shard = np.ascontiguousarray(init_state[r0:r0 + R])
        in_maps.append(make_inputs(z_shard, init_shard, T, NB, dt))
    res = run_bass_kernel_spmd(nc, in_maps, core_ids=list(range(NCORES)),
                               trace=_trace)
    hist = np.empty((NS, NR, 5), np.float32)
    for cix in range(NCORES):
        r0 = cix * R
        hist[:, r0:r0 + R, :] = (
            res.results[cix]["hist"].reshape(NB * T, R, 5)[:NS])
    state_vals = np.ascontiguousarray(hist[-1, :, :4])
    if _trace:
        return (state_vals, hist), res
    return state_vals, hist


# revision 4
# speedup vs baseline: 7407.4538x; 7407.4538x over previous
"""Bass kernel for the Balloon-Windkessel BOLD layer (trn2, 8 cores).

Layout: time-on-partitions (T=128 rows per block), regions on free dim.
Per core: R regions (250), NB blocks of T steps (time padded to NB*T).

Per block (old-aligned XO,FO,VO; outputs new-aligned):
  XO/FO : bf16-split-3 matmuls over Z + f32 1-row carry matmuls
  XN/FN : elementwise from XO,FO,Z
  v     : strict-cumsum base + ramp guess + M delta-Picard iterations
          (iteration deltas in bf16, accumulated in one PSUM group)
  q     : exact product-form solve (poly ln(alpha), exp cumsums, split-2)
  bold  : elementwise; 1/v, 1/f via DVE reciprocal
"""
import numpy as np
import ml_dtypes

import concourse.bacc as bacc
import concourse.mybir as mybir
from concourse.tile import TileContext

F32 = mybir.dt.float32
BF16 = mybir.dt.bfloat16
NPBF = ml_dtypes.bfloat16
AF = mybir.ActivationFunctionType
OP = mybir.AluOpType

KAPPA, GAMMAB, TAO, ALPHA, RO = 0.65, 0.41, 0.98, 0.32, 0.34
V_0, K_1, K_2, K_3 = 0.02, 2.38, 2.0, 0.48


def host_consts(T, dt):
    """All block constants, computed in float64, split for the PE."""
    a = 1.0 - dt * KAPPA
    b = dt * GAMMAB
    c = dt / TAO
    M2 = np.array([[a, -b], [dt, 1.0]])
    A = np.zeros((T + 1, 2, 2))
    A[0] = np.eye(2)
    for p in range(1, T + 1):
        A[p] = M2 @ A[p - 1]
    LXZ = np.zeros((T, T))
    LFZ = np.zeros((T, T))
    for p in range(T):
        for j in range(p):
            LXZ[j, p] = dt * A[p - 1 - j, 0, 0]
            LFZ[j, p] = dt * A[p - 1 - j, 1, 0]
    cum11 = np.cumsum(A[:T, 0, 0])
    cum21 = np.cumsum(A[:T, 1, 0])
    cxo0 = np.zeros(T)
    cfo0 = np.zeros(T)
    cxo0[1:] = b * cum11[:T - 1]
    cfo0[1:] = b * cum21[:T - 1]

    def bs(x):
        hi = x.astype(NPBF)
        lo = (x - hi.astype(np.float64)).astype(NPBF)
        return hi, lo

    LXZh, LXZl = bs(LXZ)
    LFZh, LFZl = bs(LFZ)
    arows = np.zeros((7, T), np.float32)
    arows[0] = A[:T, 0, 0]
    arows[1] = A[:T, 0, 1]
    arows[2] = A[:T, 1, 0]
    arows[3] = A[:T, 1, 1]
    arows[4] = -np.arange(T) / c          # ramp (times gc row, scaled by c later)
    arows[5] = 1.0 / c                    # vc broadcast (scaled by c later)
    arows[6] = 1.0                        # qc broadcast
    cols = np.zeros((T, 3), np.float32)
    cols[:, 0] = a * cxo0 + b
    cols[:, 1] = dt * cxo0
    cols[:, 2] = cfo0
    return dict(
        LXZh=LXZh, LXZl=LXZl, LFZh=LFZh, LFZl=LFZl,
        SS01=np.triu(np.ones((T, T)), 1).astype(NPBF),
        SI01=np.triu(np.ones((T, T)), 0).astype(NPBF),
        NSI01=(-np.triu(np.ones((T, T)), 0)).astype(NPBF),
        arows=arows, cols=cols,
    )


def make_inputs(z_shard, init_shard, T, NB, dt):
    """Build the per-core in_map. z_shard [NS, R] f32, init_shard [R, 4]."""
    NS, R = z_shard.shape
    c = dt / TAO
    ia = 1.0 / ALPHA
    z = np.zeros((NB * T, R), np.float32)
    z[:NS] = z_shard
    C = host_consts(T, dt)
    init4 = np.ascontiguousarray(init_shard.T.astype(np.float32))  # [4, R]
    gc0 = (c * np.exp(ia * np.log(init4[2].astype(np.float64)))
           ).astype(np.float32)[None, :]
    return {
        "z": z, "init4": init4, "gc0": gc0,
        "lxzh": C["LXZh"], "lxzl": C["LXZl"],
        "lfzh": C["LFZh"], "lfzl": C["LFZl"],
        "ss01": C["SS01"], "si01": C["SI01"], "nsi01": C["NSI01"],
        "arows": C["arows"], "cols": C["cols"],
    }


def build_kernel(R, T, NB, M, dt, opts=()):
    opts = set(opts)
    c = dt / TAO
    ia = 1.0 / ALPHA
    lnc = float(np.log(c))
    lncRO = float(np.log(c / RO))
    lnrho = float(np.log(1.0 - RO))
    a = 1.0 - dt * KAPPA
    b = dt * GAMMAB
    k1, k2, k3 = V_0 * K_1, V_0 * K_2, V_0 * K_3
    CB = V_0 * (K_1 + K_2 + K_3)

    nc = bacc.Bacc("TRN2", target_bir_lowering=False, debug=False)
    z_d = nc.dram_tensor("z", [NB * T, R], F32, kind="ExternalInput")
    init_d = nc.dram_tensor("init4", [4, R], F32, kind="ExternalInput")
    gc0_d = nc.dram_tensor("gc0", [1, R], F32, kind="ExternalInput")
    lxzh_d = nc.dram_tensor("lxzh", [T, T], BF16, kind="ExternalInput")
    lxzl_d = nc.dram_tensor("lxzl", [T, T], BF16, kind="ExternalInput")
    lfzh_d = nc.dram_tensor("lfzh", [T, T], BF16, kind="ExternalInput")
    lfzl_d = nc.dram_tensor("lfzl", [T, T], BF16, kind="ExternalInput")
    ss_d = nc.dram_tensor("ss01", [T, T], BF16, kind="ExternalInput")
    si_d = nc.dram_tensor("si01", [T, T], BF16, kind="ExternalInput")
    nsi_d = nc.dram_tensor("nsi01", [T, T], BF16, kind="ExternalInput")
    ar_d = nc.dram_tensor("arows", [7, T], F32, kind="ExternalInput")
    cols_d = nc.dram_tensor("cols", [T, 3], F32, kind="ExternalInput")
    hist_d = nc.dram_tensor("hist", [NB * T, R * 5], F32, kind="ExternalOutput")

    with TileContext(nc) as tc:
        with (
            tc.tile_pool(name="cst", bufs=1) as cst,
            tc.tile_pool(name="zp", bufs=4) as zp,
            tc.tile_pool(name="wk", bufs=3) as wk,
            tc.tile_pool(name="gh", bufs=3) as ghp,
            tc.tile_pool(name="op", bufs=3) as op,
            tc.tile_pool(name="ps", bufs=1, space="PSUM") as ps,
            tc.tile_pool(name="ps2", bufs=2, space="PSUM") as ps2,
        ):
            if "psum2" in opts:
                ps_px = ps_pf = ps_sq = ps_pc = ps2
            else:
                ps_px = ps_pf = ps_sq = ps_pc = ps
            LXZh = cst.tile([T, T], BF16)
            LXZl = cst.tile([T, T], BF16)
            LFZh = cst.tile([T, T], BF16)
            LFZl = cst.tile([T, T], BF16)
            SS = cst.tile([T, T], BF16)
            SI = cst.tile([T, T], BF16)
            NSI = cst.tile([T, T], BF16)
            AR = cst.tile([7, T], F32)
            COLS = cst.tile([T, 3], F32)
            INIT = cst.tile([4, R], F32)
            GC0 = cst.tile([1, R], F32)
            for t_, d_ in ((LXZh, lxzh_d), (LXZl, lxzl_d), (LFZh, lfzh_d),
                           (LFZl, lfzl_d), (SS, ss_d), (SI, si_d),
                           (NSI, nsi_d), (AR, ar_d), (COLS, cols_d),
                           (INIT, init_d), (GC0, gc0_d)):
                nc.sync.dma_start(t_[:, :], d_[:, :])

            xc = INIT[0:1, :]
            fc = INIT[1:2, :]
            vc = INIT[2:3, :]
            qc = INIT[3:4, :]
            gc = GC0[0:1, :]

            for k in range(NB):
                Z = zp.tile([T, R], F32, tag="z")
                nc.sync.dma_start(Z[:, :], z_d[k * T:(k + 1) * T, :])
                Zh = zp.tile([T, R], BF16, tag="zh")
                nc.gpsimd.tensor_copy(Zh[:, :], Z[:, :])
                Zl = zp.tile([T, R], BF16, tag="zl")
                nc.vector.scalar_tensor_tensor(
                    Zl[:, :], Zh[:, :], -1.0, Z[:, :], OP.mult, OP.add)

                PX = ps_px.tile([T, R], F32, tag="px")
                nc.tensor.matmul(PX[:, :], LXZh[:, :], Zh[:, :], start=True, stop=False)
                nc.tensor.matmul(PX[:, :], LXZh[:, :], Zl[:, :], start=False, stop=False)
                nc.tensor.matmul(PX[:, :], LXZl[:, :], Zh[:, :], start=False, stop=False)
                nc.tensor.matmul(PX[:, :], AR[0:1, :], xc, start=False, stop=False)
                nc.tensor.matmul(PX[:, :], AR[1:2, :], fc, start=False, stop=True)
                PF = ps_pf.tile([T, R], F32, tag="pf")
                nc.tensor.matmul(PF[:, :], LFZh[:, :], Zh[:, :], start=True, stop=False)
                nc.tensor.matmul(PF[:, :], LFZh[:, :], Zl[:, :], start=False, stop=False)
                nc.tensor.matmul(PF[:, :], LFZl[:, :], Zh[:, :], start=False, stop=False)
                nc.tensor.matmul(PF[:, :], AR[2:3, :], xc, start=False, stop=False)
                nc.tensor.matmul(PF[:, :], AR[3:4, :], fc, start=False, stop=True)

                FOs = wk.tile([T, R], F32, tag="fos")
                nc.scalar.activation(FOs[:, :], PF[:, :], AF.Identity,
                                     bias=COLS[:, 2:3], scale=1.0)

                OUT = op.tile([T, R * 5], F32, tag="out")
                O3 = OUT[:, :].rearrange("p (r c) -> p r c", c=5)
                t0 = wk.tile([T, R], F32, tag="t0")
                nc.scalar.activation(t0[:, :], Z[:, :], AF.Identity,
                                     bias=COLS[:, 0:1], scale=dt)
                t1 = wk.tile([T, R], F32, tag="t1")
                nc.vector.scalar_tensor_tensor(
                    t1[:, :], FOs[:, :], -b, t0[:, :], OP.mult, OP.add)
                nc.vector.scalar_tensor_tensor(
                    O3[:, :, 0], PX[:, :], a, t1[:, :], OP.mult, OP.add)
                t3 = wk.tile([T, R], F32, tag="t3")
                nc.scalar.activation(t3[:, :], PX[:, :], AF.Identity,
                                     bias=COLS[:, 1:2], scale=dt)
                nc.gpsimd.tensor_tensor(O3[:, :, 1], t3[:, :], FOs[:, :], OP.add)

                FOh = zp.tile([T, R], BF16, tag="foh")
                nc.gpsimd.tensor_copy(FOh[:, :], FOs[:, :])
                FOl = zp.tile([T, R], BF16, tag="fol")
                nc.vector.scalar_tensor_tensor(
                    FOl[:, :], FOh[:, :], -1.0, FOs[:, :], OP.mult, OP.add)
                PB = ps.tile([T, R], F32, tag="pb")
                nc.tensor.matmul(PB[:, :], SS[:, :], FOh[:, :], start=True, stop=False)
                nc.tensor.matmul(PB[:, :], SS[:, :], FOl[:, :], start=False, stop=False)
                nc.tensor.matmul(PB[:, :], AR[5:6, :], vc, start=False, stop=False)
                base_sb = wk.tile([T, R], F32, tag="base")
                nc.scalar.mul(base_sb[:, :], PB[:, :], c)
                nc.tensor.matmul(PB[:, :], AR[4:5, :], gc, start=False, stop=True)

                lnV = wk.tile([T, R], F32, tag="lnv")
                nc.scalar.activation(lnV[:, :], PB[:, :], AF.Log, scale=c)
                GHprev = None
                PG = None
                VO = None
                for m in range(M):
                    GH = ghp.tile([T, R], F32, tag="gh")
                    nc.scalar.activation(GH[:, :], lnV[:, :], AF.Exp,
                                         bias=lnc, scale=ia)
                    if m == 0:
                        GHh = zp.tile([T, R], BF16, tag="ghh")
                        nc.gpsimd.tensor_copy(GHh[:, :], GH[:, :])
                        GHl = zp.tile([T, R], BF16, tag="ghl")
                        nc.vector.scalar_tensor_tensor(
                            GHl[:, :], GHh[:, :], -1.0, GH[:, :], OP.mult, OP.add)
                        PG = ps.tile([T, R], F32, tag="pg")
                        nc.tensor.matmul(PG[:, :], SS[:, :], GHh[:, :],
                                         start=True, stop=False)
                        nc.tensor.matmul(PG[:, :], SS[:, :], GHl[:, :],
                                         start=False, stop=(M == 1))
                    else:
                        D = zp.tile([T, R], BF16, tag="d")
                        nc.vector.scalar_tensor_tensor(
                            D[:, :], GHprev[:, :], -1.0, GH[:, :], OP.mult, OP.add)
                        nc.tensor.matmul(PG[:, :], SS[:, :], D[:, :],
                                         start=False, stop=(m == M - 1))
                    GHprev = GH
                    VO = wk.tile([T, R], F32, tag="vo")
                    nc.vector.scalar_tensor_tensor(
                        VO[:, :], PG[:, :], -1.0, base_sb[:, :], OP.mult, OP.add)
                    if m < M - 1:
                        lnV = wk.tile([T, R], F32, tag="lnv")
                        nc.scalar.activation(lnV[:, :], VO[:, :], AF.Log)

                tv = wk.tile([T, R], F32, tag="tv")
                nc.vector.scalar_tensor_tensor(
                    tv[:, :], GHprev[:, :], -1.0, VO[:, :], OP.mult, OP.add)
                nc.vector.scalar_tensor_tensor(
                    O3[:, :, 2], FOs[:, :], c, tv[:, :], OP.mult, OP.add)

                Y = wk.tile([T, R], F32, tag="y")
                nc.scalar.activation(Y[:, :], lnV[:, :], AF.Exp,
                                     bias=lnc, scale=ia - 1.0)
                s1 = wk.tile([T, R], F32, tag="s1")
                nc.vector.tensor_scalar(s1[:, :], Y[:, :], 1.0 / 3.0, 0.5,
                                        OP.mult, OP.add)
                s2 = wk.tile([T, R], F32, tag="s2")
                nc.gpsimd.tensor_tensor(s2[:, :], Y[:, :], s1[:, :], OP.mult)
                s3 = wk.tile([T, R], F32, tag="s3")
                nc.vector.scalar_tensor_tensor(
                    s3[:, :], s2[:, :], 1.0, Y[:, :], OP.add, OP.mult)
                s3h = zp.tile([T, R], BF16, tag="s3h")
                nc.gpsimd.tensor_copy(s3h[:, :], s3[:, :])
                s3l = zp.tile([T, R], BF16, tag="s3l")
                nc.vector.scalar_tensor_tensor(
                    s3l[:, :], s3h[:, :], -1.0, s3[:, :], OP.mult, OP.add)
                PS = ps_sq.tile([T, R], F32, tag="psq")
                nc.tensor.matmul(PS[:, :], NSI[:, :], s3h[:, :], start=True, stop=False)
                nc.tensor.matmul(PS[:, :], NSI[:, :], s3l[:, :], start=False, stop=True)
                P_sb = wk.tile([T, R], F32, tag="p")
                nc.scalar.activation(P_sb[:, :], PS[:, :], AF.Exp)
                E_sb = wk.tile([T, R], F32, tag="e")
                nc.scalar.activation(E_sb[:, :], PS[:, :], AF.Exp,
                                     bias=lncRO, scale=-1.0)
                RF = wk.tile([T, R], F32, tag="rfq")
                nc.vector.reciprocal(RF[:, :], FOs[:, :])
                PW = wk.tile([T, R], F32, tag="pw")
                nc.scalar.activation(PW[:, :], RF[:, :], AF.Exp, scale=lnrho)
                tq = wk.tile([T, R], F32, tag="tq")
                nc.gpsimd.tensor_tensor(tq[:, :], FOs[:, :], PW[:, :], OP.mult)
                Rm = wk.tile([T, R], F32, tag="rmq")
                nc.vector.scalar_tensor_tensor(
                    Rm[:, :], tq[:, :], -1.0, FOs[:, :], OP.mult, OP.add)
                RMf = wk.tile([T, R], F32, tag="rmf")
                nc.gpsimd.tensor_tensor(RMf[:, :], Rm[:, :], E_sb[:, :], OP.mult)
                RMh = zp.tile([T, R], BF16, tag="rmh")
                nc.gpsimd.tensor_copy(RMh[:, :], RMf[:, :])
                RMl = zp.tile([T, R], BF16, tag="rml")
                nc.vector.scalar_tensor_tensor(
                    RMl[:, :], RMh[:, :], -1.0, RMf[:, :], OP.mult, OP.add)
                PC = ps_pc.tile([T, R], F32, tag="pc")
                nc.tensor.matmul(PC[:, :], SI[:, :], RMh[:, :], start=True, stop=False)
                nc.tensor.matmul(PC[:, :], SI[:, :], RMl[:, :], start=False, stop=False)
                nc.tensor.matmul(PC[:, :], AR[6:7, :], qc, start=False, stop=True)
                nc.vector.tensor_tensor(O3[:, :, 3], P_sb[:, :], PC[:, :], OP.mult)

                RV = wk.tile([T, R], F32, tag="rvb")
                nc.vector.reciprocal(RV[:, :], O3[:, :, 2])
                t2 = wk.tile([T, R], F32, tag="t2")
                nc.vector.scalar_tensor_tensor(
                    t2[:, :], RV[:, :], k2, O3[:, :, 3], OP.mult, OP.mult)
                w1 = wk.tile([T, R], F32, tag="w1")
                nc.vector.scalar_tensor_tensor(
                    w1[:, :], O3[:, :, 3], k1, t2[:, :], OP.mult, OP.add)
                w2 = wk.tile([T, R], F32, tag="w2")
                nc.vector.scalar_tensor_tensor(
                    w2[:, :], O3[:, :, 2], k3, w1[:, :], OP.mult, OP.add)
                nc.scalar.activation(O3[:, :, 4], w2[:, :], AF.Identity,
                                     bias=CB, scale=-1.0)

                nc.sync.dma_start(hist_d[k * T:(k + 1) * T, :], OUT[:, :])

                xc = O3[T - 1:T, :, 0]
                fc = O3[T - 1:T, :, 1]
                vc = O3[T - 1:T, :, 2]
                qc = O3[T - 1:T, :, 3]
                gc = GHprev[T - 1:T, :]
    nc.compile()
    return nc


_BUILD_CACHE = {}


def _get_kernel(R, T, NB, M, dt):
    key = (R, T, NB, M, dt)
    if key not in _BUILD_CACHE:
        _BUILD_CACHE[key] = build_kernel(R, T, NB, M, dt)
    return _BUILD_CACHE[key]


def kernel(init_state, node_history, step_size, sim_len, _trace=False):
    """Full-input entry point: shards regions over 8 neuron cores."""
    from concourse.bass_utils import run_bass_kernel_spmd

    init_state = np.asarray(init_state, dtype=np.float32)
    node_history = np.asarray(node_history, dtype=np.float32)
    step_size = int(step_size)
    sim_len = int(sim_len)
    NS = sim_len // step_size
    assert node_history.shape[0] == NS
    NR = node_history.shape[1]
    NCORES = 8
    assert NR % NCORES == 0
    R = NR // NCORES
    T = 128
    NB = (NS + T - 1) // T
    M = 3
    dt = step_size / 1000.0

    nc = _get_kernel(R, T, NB, M, dt)
    in_maps = []
    for cix in range(NCORES):
        r0 = cix * R
        z_shard = np.ascontiguousarray(node_history[:, r0:r0 + R])
        init_shard = np.ascontiguousarray(init_state[r0:r0 + R])
        in_maps.append(make_inputs(z_shard, init_shard, T, NB, dt))
    res = run_bass_kernel_spmd(nc, in_maps, core_ids=list(range(NCORES)),
                               trace=_trace)
    hist = np.empty((NS, NR, 5), np.float32)
    for cix in range(NCORES):
        r0 = cix * R
        hist[:, r0:r0 + R, :] = (
            res.results[cix]["hist"].reshape(NB * T, R, 5)[:NS])
    state_vals = np.ascontiguousarray(hist[-1, :, :4])
    if _trace:
        return (state_vals, hist), res
    return state_vals, hist
